# revision 16
# baseline (speedup 1.0000x reference)
"""Bass/Trainium2 kernel for nn_Bilinear (out[b,n,i] = enc[b,n,i,:] @ W @ hidden[b,:] + bias).

Sharding: data-parallel over B. 8 cores, one batch element each.

The kernel is HBM-traffic-bound (enc is 32 MiB/core in fp32), so all streamed
operands are cast to fp16 on the host (harness gate is rel_err < 2e-2; fp16
lands ~4e-4): enc 16 MiB + W 2 MiB per core.

With fp16 the DMA stream (~44 us at the observed ~420 GB/s per-core rate)
outpaces what DVE+ScalarE alone can compute (~55 us), so stage 2 is split
across THREE engines. The host lays out each 2 MiB chunk of enc as:
  [ 4 row-major blocks | same-size j-major (transposed) strip of 4 blocks ]
so every DMA is one fully-contiguous 16 KiB run per partition, and:
  - the j-major strip half is reduced on the otherwise-idle TensorE as 8
    PSUM-accumulated [K=128]x[1,512] matmuls against v_col (~2.8 us/chunk),
  - 2 row-major blocks go to DVE custom TENSOR_TENSOR_REDUCE (~1.1 us each),
  - 2 row-major blocks go to DVE fp16 tensor_mul (2x mode, ~0.6 us) +
    ScalarE accumulate-Copy (~1.2 us each).
Per chunk each engine needs ~3 us vs ~4.9 us of DMA: DMA-paced end to end.

Stage 1 (TensorE, fp16): v = W @ h via 16 PSUM-accumulated matmuls behind 8
chunked wt DMAs; v is then both partition-broadcast on the PE (v_rep, for
DVE/ScalarE) and PE-transposed 128 at a time into column form (v_col, the
matmul stationary operand). Bias is folded into the strip PSUM-drain
(activation bias) and added to the block-accumulated columns at the end.

Host-side prep is layout/dtype only (transpose/cast/reshape); all arithmetic
runs on device. The host re-assembles the three output tensors (strip rows,
TTR columns, mul+accum columns) into the full [B, N, I] output.
"""

import numpy as np

B, N, I, H = 8, 64, 128, 1024
P = 128
NI = N * I  # 8192 rows per core
KB = H // P  # 8 k blocks for stage 1
N_CORES = 8
NCH = 8  # stage-2 chunks per core; each covers 8 row blocks (2 MiB fp16)

_NC_CACHE = {}
LAST_RESULTS = None


def _build(ebufs=6):
    import concourse.bacc as bacc
    import concourse.mybir as mybir
    import concourse.tile as tile
    from concourse import dve_ops

    f32 = mybir.dt.float32
    f16 = mybir.dt.float16
    Copy = mybir.ActivationFunctionType.Copy

    nc = bacc.Bacc(
        "TRN2",
        target_bir_lowering=False,
        debug=False,
        num_devices=N_CORES,
    )
    enc = nc.declare_dram_parameter("enc", [P, NCH * 8192], f16, isOutput=False)
    hh = nc.declare_dram_parameter("h", [P, KB], f16, isOutput=False)
    wt = nc.declare_dram_parameter("wt", [P, KB * H], f16, isOutput=False)
    bb = nc.declare_dram_parameter("bias", [1, 1], f32, isOutput=False)
    out_rows = nc.declare_dram_parameter("out_rows", [1, NCH * 512], f32, isOutput=True)
    out_a = nc.declare_dram_parameter("out_a", [P, 2 * NCH], f32, isOutput=True)
    out_b = nc.declare_dram_parameter("out_b", [P, 2 * NCH], f32, isOutput=True)

    with tile.TileContext(nc) as tc:
        with (
            tc.tile_pool(name="const", bufs=1) as const,
            tc.tile_pool(name="tpool", bufs=ebufs) as tpool,
            tc.tile_pool(name="rpool", bufs=ebufs) as rpool,
            tc.tile_pool(name="ppool", bufs=3) as ppool,
            tc.tile_pool(name="vpsum", bufs=1, space="PSUM") as vpsum,
            tc.tile_pool(name="spsum", bufs=3, space="PSUM") as spsum,
        ):
            # ---- stage 1: v[j] = sum_k wt[k,j] h[k] ----
            h_col = const.tile([P, KB], f16)
            nc.sync.dma_start(out=h_col[:], in_=hh[:, :])
            bias_col = const.tile([P, 1], f32)
            nc.sync.dma_start(out=bias_col[:], in_=bb[:, :].to_broadcast((P, 1)))
            bias_one = const.tile([1, 1], f32)
            nc.sync.dma_start(out=bias_one[:], in_=bb[:, :])
            # wt host-packed as [p, kb*H + j] = W.T[kb*128+p, j]: one DMA,
            # 16 KiB contiguous per partition
            wt_sb = const.tile([P, KB * H], f16)
            nc.sync.dma_start(out=wt_sb[:], in_=wt[:, :])
            ones = const.tile([1, P], f16)
            nc.vector.memset(ones[:], 1.0)
            id1 = const.tile([1, 1], f16)
            nc.vector.memset(id1[:], 1.0)

            v_flat = const.tile([1, H], f16)
            vps = [
                vpsum.tile([1, 512], f32, name=f"vp{jc}", tag=f"vp{jc}")
                for jc in range(H // 512)
            ]
            for kb in range(KB):
                for jc in range(H // 512):
                    nc.tensor.matmul(
                        vps[jc][:],
                        h_col[:, kb : kb + 1],
                        wt_sb[:, kb * H + jc * 512 : kb * H + (jc + 1) * 512],
                        start=(kb == 0),
                        stop=(kb == KB - 1),
                    )
            for jc in range(H // 512):
                nc.scalar.activation(
                    v_flat[:, jc * 512 : (jc + 1) * 512], vps[jc][:], Copy
                )
            # partition-broadcast v on the PE: ones[1,P].T @ v[1,512] -> [P,512]
            v_rep = const.tile([P, H], f16)
            for jc in range(H // 512):
                bc = vpsum.tile([P, 512], f32, name=f"bc{jc}", tag=f"bc{jc}")
                nc.tensor.matmul(
                    bc[:],
                    ones[:],
                    v_flat[:, jc * 512 : (jc + 1) * 512],
                    start=True,
                    stop=True,
                )
                nc.scalar.activation(
                    v_rep[:, jc * 512 : (jc + 1) * 512], bc[:], Copy
                )
            # column form of v for the strip matmuls: v_col[p, jb] = v[jb*128+p]
            v_col = const.tile([P, KB], f16)
            for jb in range(KB):
                pt = vpsum.tile([P, 1], f16, name=f"pt{jb}", tag="pt")
                nc.tensor.transpose(
                    pt[:], v_flat[:, jb * P : (jb + 1) * P], id1[:]
                )
                nc.scalar.activation(v_col[:, jb : jb + 1], pt[:], Copy)

            # ---- stage 2 ----
            # Per chunk: DMA the j-major strip half first (PE consumes it),
            # then the row-major half (DVE/ScalarE). The last chunk's DMAs
            # are split in half again to shorten the trailing compute.
            acc_a = const.tile([P, 2 * NCH], f32)
            acc_b = const.tile([P, 2 * NCH], f32)
            dummy_a = const.tile([P, 1], f16)
            Ident = mybir.ActivationFunctionType.Identity

            def rm_block(e_sl, path, col):
                if path == "A":
                    nc.vector._custom_dve(
                        dve_ops.TENSOR_TENSOR_REDUCE,
                        out=dummy_a[:].broadcast_to((P, H)),
                        in0=e_sl,
                        in1=v_rep[:],
                        s0=0.0,
                        s1=1.0,
                        accum_out=acc_a[:, col : col + 1],
                    )
                else:
                    prod = ppool.tile([P, H], f16)
                    nc.vector.tensor_mul(prod[:], e_sl, v_rep[:])
                    nc.scalar.activation(
                        prod[:], prod[:], Copy, accum_out=acc_b[:, col : col + 1]
                    )

            for ci in range(NCH):
                base = ci * 8192
                last = ci == NCH - 1
                ps = spsum.tile([1, 512], f32, name=f"ps{ci}", tag="ps")
                strip = const.tile([1, 512], f32, name=f"st{ci}", tag=f"st{ci}")
                if not last:
                    t = tpool.tile([P, 4096], f16, name=f"t{ci}", tag="t")
                    nc.sync.dma_start(out=t[:], in_=enc[:, base + 4096 : base + 8192])
                    r = rpool.tile([P, 4096], f16, name=f"r{ci}", tag="r")
                    nc.sync.dma_start(out=r[:], in_=enc[:, base : base + 4096])
                    for jb in range(KB):
                        nc.tensor.matmul(
                            ps[:],
                            v_col[:, jb : jb + 1],
                            t[:, jb * 512 : (jb + 1) * 512],
                            start=(jb == 0),
                            stop=(jb == KB - 1),
                        )
                    nc.scalar.activation(strip[:], ps[:], Ident, bias=bias_one[:])
                    nc.sync.dma_start(
                        out=out_rows[:, ci * 512 : (ci + 1) * 512], in_=strip[:]
                    )
                    for slot, path in enumerate(("A", "B", "A", "B")):
                        rm_block(
                            r[:, slot * 1024 : (slot + 1) * 1024],
                            path,
                            2 * ci + slot // 2,
                        )
                else:
                    # tapered final chunk: 4 half-size DMAs
                    ta = tpool.tile([P, 2048], f16, name="t7a", tag="t")
                    nc.sync.dma_start(out=ta[:], in_=enc[:, base + 4096 : base + 6144])
                    tb = tpool.tile([P, 2048], f16, name="t7b", tag="t")
                    nc.sync.dma_start(out=tb[:], in_=enc[:, base + 6144 : base + 8192])
                    ra = rpool.tile([P, 2048], f16, name="r7a", tag="r")
                    nc.sync.dma_start(out=ra[:], in_=enc[:, base : base + 2048])
                    rb = rpool.tile([P, 2048], f16, name="r7b", tag="r")
                    nc.sync.dma_start(out=rb[:], in_=enc[:, base + 2048 : base + 4096])
                    for jb in range(KB):
                        src = ta if jb < 4 else tb
                        off = (jb % 4) * 512
                        nc.tensor.matmul(
                            ps[:],
                            v_col[:, jb : jb + 1],
                            src[:, off : off + 512],
                            start=(jb == 0),
                            stop=(jb == KB - 1),
                        )
                    nc.scalar.activation(strip[:], ps[:], Ident, bias=bias_one[:])
                    nc.sync.dma_start(
                        out=out_rows[:, ci * 512 : (ci + 1) * 512], in_=strip[:]
                    )
                    rm_block(ra[:, 0:1024], "A", 2 * ci)
                    rm_block(ra[:, 1024:2048], "B", 2 * ci)
                    rm_block(rb[:, 0:1024], "A", 2 * ci + 1)
                    rm_block(rb[:, 1024:2048], "B", 2 * ci + 1)

            # bias + writeback of the block-accumulated columns: head columns
            # overlap the final chunk's compute, only the last 2 stay serial
            head = 2 * NCH - 2
            nc.vector.tensor_scalar_add(
                acc_a[:, :head], acc_a[:, :head], bias_col[:]
            )
            nc.sync.dma_start(out=out_a[:, :head], in_=acc_a[:, :head])
            nc.vector.tensor_scalar_add(
                acc_b[:, :head], acc_b[:, :head], bias_col[:]
            )
            nc.sync.dma_start(out=out_b[:, :head], in_=acc_b[:, :head])
            nc.vector.tensor_scalar_add(
                acc_a[:, head:], acc_a[:, head:], bias_col[:]
            )
            nc.sync.dma_start(out=out_a[:, head:], in_=acc_a[:, head:])
            nc.vector.tensor_scalar_add(
                acc_b[:, head:], acc_b[:, head:], bias_col[:]
            )
            nc.sync.dma_start(out=out_b[:, head:], in_=acc_b[:, head:])
    nc.compile()
    return nc


def _get_nc():
    if "nc" not in _NC_CACHE:
        _NC_CACHE["nc"] = _build()
    return _NC_CACHE["nc"]


def _pack_enc(enc16_core):
    """[N*I, H] fp16 -> [P, NCH*8192]: per chunk ci, 4 row-major blocks
    (8ci+4..8ci+7) then the j-major strip of blocks 8ci+0..8ci+3."""
    E = enc16_core.reshape(NCH, 8, P, H)  # [ci, slot, i, j], blk = 8ci+slot
    rm = E[:, 4:8]  # [ci, slot, i, j]
    rm_part = rm.transpose(2, 0, 1, 3).reshape(P, NCH, 4096)  # [i, ci, slot*H+j]
    tr = E[:, 0:4].reshape(NCH, 4, P, KB, P)  # [ci, slot, i, jb, p]
    tr_part = tr.transpose(4, 0, 3, 1, 2).reshape(P, NCH, 4096)  # [p, ci, jb*512+slot*128+i]
    comb = np.concatenate([rm_part, tr_part], axis=2)  # [P, NCH, 8192]
    return np.ascontiguousarray(comb.reshape(P, NCH * 8192))


def kernel(hidden=None, encoder_hiddens=None, input_lengths=None, W=None, b=None):
    global LAST_RESULTS
    from concourse.bass_utils import run_bass_kernel_spmd

    hidden = np.asarray(hidden, dtype=np.float32)
    enc = np.asarray(encoder_hiddens, dtype=np.float32)
    W_ = np.asarray(W, dtype=np.float32)
    b_ = np.asarray(b, dtype=np.float32).reshape(1, 1)
    # wt packed [p, kb*H + j] = W.T[kb*128+p, j]: one contiguous-run DMA
    wt16 = np.ascontiguousarray(
        W_.T.astype(np.float16).reshape(KB, P, H).transpose(1, 0, 2).reshape(P, KB * H)
    )
    enc16 = enc.astype(np.float16)  # [B, N, I, H]

    nc = _get_nc()
    in_maps = []
    for core in range(N_CORES):
        in_maps.append(
            {
                "enc": _pack_enc(enc16[core].reshape(NI, H)),
                "h": np.ascontiguousarray(
                    hidden[core].reshape(KB, P).T.astype(np.float16)
                ),
                "wt": wt16,
                "bias": b_,
            }
        )
    res = run_bass_kernel_spmd(nc, in_maps, core_ids=list(range(N_CORES)))
    LAST_RESULTS = res

    out = np.empty((N_CORES, N, P), dtype=np.float32)
    for c in range(N_CORES):
        r = res.results[c]
        strips = np.asarray(r["out_rows"], dtype=np.float32).reshape(NCH, 4, P)
        a_cols = np.asarray(r["out_a"], dtype=np.float32).T.reshape(NCH, 2, P)
        b_cols = np.asarray(r["out_b"], dtype=np.float32).T.reshape(NCH, 2, P)
        O = out[c].reshape(NCH, 8, P)
        O[:, 0:4] = strips
        O[:, 4] = a_cols[:, 0]
        O[:, 6] = a_cols[:, 1]
        O[:, 5] = b_cols[:, 0]
        O[:, 7] = b_cols[:, 1]
    return np.ascontiguousarray(out)


# revision 17
# speedup vs baseline: 1.0730x; 1.0730x over previous
"""Bass/Trainium2 kernel for nn_Bilinear (out[b,n,i] = enc[b,n,i,:] @ W @ hidden[b,:] + bias).

Sharding: data-parallel over B. 8 cores, one batch element each.

The kernel is HBM-traffic-bound (enc is 32 MiB/core in fp32), so all streamed
operands are cast to fp16 on the host (harness gate is rel_err < 2e-2; fp16
lands ~4e-4): enc 16 MiB + W 2 MiB per core.

With fp16 the DMA stream (~44 us at the observed ~420 GB/s per-core rate)
outpaces what DVE+ScalarE alone can compute (~55 us), so stage 2 is split
across THREE engines. The host lays out each 2 MiB chunk of enc as:
  [ 4 row-major blocks | same-size j-major (transposed) strip of 4 blocks ]
so every DMA is one fully-contiguous 16 KiB run per partition, and:
  - the j-major strip half is reduced on the otherwise-idle TensorE as 8
    PSUM-accumulated [K=128]x[1,512] matmuls against v_col (~2.8 us/chunk),
  - 2 row-major blocks go to DVE custom TENSOR_TENSOR_REDUCE (~1.1 us each),
  - 2 row-major blocks go to DVE fp16 tensor_mul (2x mode, ~0.6 us) +
    ScalarE accumulate-Copy (~1.2 us each).
Per chunk each engine needs ~3 us vs ~4.9 us of DMA: DMA-paced end to end.

Stage 1 (TensorE, fp16): v = W @ h via 16 PSUM-accumulated matmuls behind 8
chunked wt DMAs; v is then both partition-broadcast on the PE (v_rep, for
DVE/ScalarE) and PE-transposed 128 at a time into column form (v_col, the
matmul stationary operand). Bias is folded into the strip PSUM-drain
(activation bias) and added to the block-accumulated columns at the end.

Host-side prep is layout/dtype only (transpose/cast/reshape); all arithmetic
runs on device. The host re-assembles the three output tensors (strip rows,
TTR columns, mul+accum columns) into the full [B, N, I] output.
"""

import numpy as np

B, N, I, H = 8, 64, 128, 1024
P = 128
NI = N * I  # 8192 rows per core
KB = H // P  # 8 k blocks for stage 1
N_CORES = 8
NCH = 8  # stage-2 chunks per core; each covers 8 row blocks (2 MiB fp16)

_NC_CACHE = {}
LAST_RESULTS = None


def _build(ebufs=6):
    import concourse.bacc as bacc
    import concourse.mybir as mybir
    import concourse.tile as tile
    from concourse import dve_ops

    f32 = mybir.dt.float32
    f16 = mybir.dt.float16
    Copy = mybir.ActivationFunctionType.Copy

    nc = bacc.Bacc(
        "TRN2",
        target_bir_lowering=False,
        debug=False,
        num_devices=N_CORES,
    )
    enc = nc.declare_dram_parameter("enc", [P, NCH * 8192], f16, isOutput=False)
    hh = nc.declare_dram_parameter("h", [P, KB], f16, isOutput=False)
    wt = nc.declare_dram_parameter("wt", [P, KB * H], f16, isOutput=False)
    bb = nc.declare_dram_parameter("bias", [1, 1], f32, isOutput=False)
    out_rows = nc.declare_dram_parameter("out_rows", [1, NCH * 512], f32, isOutput=True)
    out_a = nc.declare_dram_parameter("out_a", [P, 2 * NCH], f32, isOutput=True)
    out_b = nc.declare_dram_parameter("out_b", [P, 2 * NCH], f32, isOutput=True)

    with tile.TileContext(nc) as tc:
        with (
            tc.tile_pool(name="const", bufs=1) as const,
            tc.tile_pool(name="tpool", bufs=ebufs) as tpool,
            tc.tile_pool(name="rpool", bufs=ebufs) as rpool,
            tc.tile_pool(name="ppool", bufs=3) as ppool,
            tc.tile_pool(name="vpsum", bufs=1, space="PSUM") as vpsum,
            tc.tile_pool(name="spsum", bufs=3, space="PSUM") as spsum,
        ):
            # ---- stage 1: v[j] = sum_k wt[k,j] h[k] ----
            h_col = const.tile([P, KB], f16)
            nc.sync.dma_start(out=h_col[:], in_=hh[:, :])
            bias_col = const.tile([P, 1], f32)
            nc.sync.dma_start(out=bias_col[:], in_=bb[:, :].to_broadcast((P, 1)))
            bias_one = const.tile([1, 1], f32)
            nc.sync.dma_start(out=bias_one[:], in_=bb[:, :])
            # wt host-packed as [p, kb*H + j] = W.T[kb*128+p, j]: one DMA,
            # 16 KiB contiguous per partition
            wt_sb = const.tile([P, KB * H], f16)
            nc.sync.dma_start(out=wt_sb[:], in_=wt[:, :])
            ones = const.tile([1, P], f16)
            nc.vector.memset(ones[:], 1.0)
            id1 = const.tile([1, 1], f16)
            nc.vector.memset(id1[:], 1.0)

            v_flat = const.tile([1, H], f16)
            vps = [
                vpsum.tile([1, 512], f32, name=f"vp{jc}", tag=f"vp{jc}")
                for jc in range(H // 512)
            ]
            for kb in range(KB):
                for jc in range(H // 512):
                    nc.tensor.matmul(
                        vps[jc][:],
                        h_col[:, kb : kb + 1],
                        wt_sb[:, kb * H + jc * 512 : kb * H + (jc + 1) * 512],
                        start=(kb == 0),
                        stop=(kb == KB - 1),
                    )
            for jc in range(H // 512):
                nc.scalar.activation(
                    v_flat[:, jc * 512 : (jc + 1) * 512], vps[jc][:], Copy
                )
            # partition-broadcast v on the PE: ones[1,P].T @ v[1,512] -> [P,512]
            v_rep = const.tile([P, H], f16)
            for jc in range(H // 512):
                bc = vpsum.tile([P, 512], f32, name=f"bc{jc}", tag=f"bc{jc}")
                nc.tensor.matmul(
                    bc[:],
                    ones[:],
                    v_flat[:, jc * 512 : (jc + 1) * 512],
                    start=True,
                    stop=True,
                )
                nc.scalar.activation(
                    v_rep[:, jc * 512 : (jc + 1) * 512], bc[:], Copy
                )
            # column form of v for the strip matmuls: v_col[p, jb] = v[jb*128+p]
            v_col = const.tile([P, KB], f16)
            for jb in range(KB):
                pt = vpsum.tile([P, 1], f16, name=f"pt{jb}", tag="pt")
                nc.tensor.transpose(
                    pt[:], v_flat[:, jb * P : (jb + 1) * P], id1[:]
                )
                nc.scalar.activation(v_col[:, jb : jb + 1], pt[:], Copy)

            # ---- stage 2 ----
            # Per chunk: DMA the j-major strip half first (PE consumes it),
            # then the row-major half (DVE/ScalarE). The last chunk's DMAs
            # are split in half again to shorten the trailing compute.
            acc_a = const.tile([P, 2 * NCH], f32)
            acc_b = const.tile([P, 2 * NCH], f32)
            dummy_a = const.tile([P, 1], f16)
            Ident = mybir.ActivationFunctionType.Identity

            def rm_block(e_sl, path, col):
                if path == "A":
                    nc.vector._custom_dve(
                        dve_ops.TENSOR_TENSOR_REDUCE,
                        out=dummy_a[:].broadcast_to((P, H)),
                        in0=e_sl,
                        in1=v_rep[:],
                        s0=0.0,
                        s1=1.0,
                        accum_out=acc_a[:, col : col + 1],
                    )
                else:
                    prod = ppool.tile([P, H], f16)
                    nc.vector.tensor_mul(prod[:], e_sl, v_rep[:])
                    nc.scalar.activation(
                        prod[:], prod[:], Copy, accum_out=acc_b[:, col : col + 1]
                    )

            for ci in range(NCH):
                base = ci * 8192
                last = ci == NCH - 1
                ps = spsum.tile([1, 512], f32, name=f"ps{ci}", tag="ps")
                strip = const.tile([1, 512], f32, name=f"st{ci}", tag=f"st{ci}")
                if not last:
                    t = tpool.tile([P, 4096], f16, name=f"t{ci}", tag="t")
                    nc.sync.dma_start(out=t[:], in_=enc[:, base + 4096 : base + 8192])
                    r = rpool.tile([P, 4096], f16, name=f"r{ci}", tag="r")
                    nc.sync.dma_start(out=r[:], in_=enc[:, base : base + 4096])
                    for jb in range(KB):
                        nc.tensor.matmul(
                            ps[:],
                            v_col[:, jb : jb + 1],
                            t[:, jb * 512 : (jb + 1) * 512],
                            start=(jb == 0),
                            stop=(jb == KB - 1),
                        )
                    nc.scalar.activation(strip[:], ps[:], Ident, bias=bias_one[:])
                    nc.gpsimd.dma_start(
                        out=out_rows[:, ci * 512 : (ci + 1) * 512], in_=strip[:]
                    )
                    for slot, path in enumerate(("A", "B", "A", "B")):
                        rm_block(
                            r[:, slot * 1024 : (slot + 1) * 1024],
                            path,
                            2 * ci + slot // 2,
                        )
                else:
                    # tapered final chunk: 4 half-size DMAs
                    ta = tpool.tile([P, 2048], f16, name="t7a", tag="t")
                    nc.sync.dma_start(out=ta[:], in_=enc[:, base + 4096 : base + 6144])
                    tb = tpool.tile([P, 2048], f16, name="t7b", tag="t")
                    nc.sync.dma_start(out=tb[:], in_=enc[:, base + 6144 : base + 8192])
                    ra = rpool.tile([P, 2048], f16, name="r7a", tag="r")
                    nc.sync.dma_start(out=ra[:], in_=enc[:, base : base + 2048])
                    rb = rpool.tile([P, 2048], f16, name="r7b", tag="r")
                    nc.sync.dma_start(out=rb[:], in_=enc[:, base + 2048 : base + 4096])
                    for jb in range(KB):
                        src = ta if jb < 4 else tb
                        off = (jb % 4) * 512
                        nc.tensor.matmul(
                            ps[:],
                            v_col[:, jb : jb + 1],
                            src[:, off : off + 512],
                            start=(jb == 0),
                            stop=(jb == KB - 1),
                        )
                    nc.scalar.activation(strip[:], ps[:], Ident, bias=bias_one[:])
                    nc.gpsimd.dma_start(
                        out=out_rows[:, ci * 512 : (ci + 1) * 512], in_=strip[:]
                    )
                    rm_block(ra[:, 0:1024], "A", 2 * ci)
                    rm_block(ra[:, 1024:2048], "B", 2 * ci)
                    rm_block(rb[:, 0:1024], "A", 2 * ci + 1)
                    rm_block(rb[:, 1024:2048], "B", 2 * ci + 1)

            # bias + writeback of the block-accumulated columns: head columns
            # overlap the final chunk's compute, only the last 2 stay serial
            head = 2 * NCH - 2
            nc.vector.tensor_scalar_add(
                acc_a[:, :head], acc_a[:, :head], bias_col[:]
            )
            nc.gpsimd.dma_start(out=out_a[:, :head], in_=acc_a[:, :head])
            nc.vector.tensor_scalar_add(
                acc_b[:, :head], acc_b[:, :head], bias_col[:]
            )
            nc.gpsimd.dma_start(out=out_b[:, :head], in_=acc_b[:, :head])
            nc.vector.tensor_scalar_add(
                acc_a[:, head:], acc_a[:, head:], bias_col[:]
            )
            nc.gpsimd.dma_start(out=out_a[:, head:], in_=acc_a[:, head:])
            nc.vector.tensor_scalar_add(
                acc_b[:, head:], acc_b[:, head:], bias_col[:]
            )
            nc.gpsimd.dma_start(out=out_b[:, head:], in_=acc_b[:, head:])
    nc.compile()
    return nc


def _get_nc():
    if "nc" not in _NC_CACHE:
        _NC_CACHE["nc"] = _build()
    return _NC_CACHE["nc"]


def _pack_enc(enc16_core):
    """[N*I, H] fp16 -> [P, NCH*8192]: per chunk ci, 4 row-major blocks
    (8ci+4..8ci+7) then the j-major strip of blocks 8ci+0..8ci+3."""
    E = enc16_core.reshape(NCH, 8, P, H)  # [ci, slot, i, j], blk = 8ci+slot
    rm = E[:, 4:8]  # [ci, slot, i, j]
    rm_part = rm.transpose(2, 0, 1, 3).reshape(P, NCH, 4096)  # [i, ci, slot*H+j]
    tr = E[:, 0:4].reshape(NCH, 4, P, KB, P)  # [ci, slot, i, jb, p]
    tr_part = tr.transpose(4, 0, 3, 1, 2).reshape(P, NCH, 4096)  # [p, ci, jb*512+slot*128+i]
    comb = np.concatenate([rm_part, tr_part], axis=2)  # [P, NCH, 8192]
    return np.ascontiguousarray(comb.reshape(P, NCH * 8192))


def kernel(hidden=None, encoder_hiddens=None, input_lengths=None, W=None, b=None):
    global LAST_RESULTS
    from concourse.bass_utils import run_bass_kernel_spmd

    hidden = np.asarray(hidden, dtype=np.float32)
    enc = np.asarray(encoder_hiddens, dtype=np.float32)
    W_ = np.asarray(W, dtype=np.float32)
    b_ = np.asarray(b, dtype=np.float32).reshape(1, 1)
    # wt packed [p, kb*H + j] = W.T[kb*128+p, j]: one contiguous-run DMA
    wt16 = np.ascontiguousarray(
        W_.T.astype(np.float16).reshape(KB, P, H).transpose(1, 0, 2).reshape(P, KB * H)
    )
    enc16 = enc.astype(np.float16)  # [B, N, I, H]

    nc = _get_nc()
    in_maps = []
    for core in range(N_CORES):
        in_maps.append(
            {
                "enc": _pack_enc(enc16[core].reshape(NI, H)),
                "h": np.ascontiguousarray(
                    hidden[core].reshape(KB, P).T.astype(np.float16)
                ),
                "wt": wt16,
                "bias": b_,
            }
        )
    res = run_bass_kernel_spmd(nc, in_maps, core_ids=list(range(N_CORES)))
    LAST_RESULTS = res

    out = np.empty((N_CORES, N, P), dtype=np.float32)
    for c in range(N_CORES):
        r = res.results[c]
        strips = np.asarray(r["out_rows"], dtype=np.float32).reshape(NCH, 4, P)
        a_cols = np.asarray(r["out_a"], dtype=np.float32).T.reshape(NCH, 2, P)
        b_cols = np.asarray(r["out_b"], dtype=np.float32).T.reshape(NCH, 2, P)
        O = out[c].reshape(NCH, 8, P)
        O[:, 0:4] = strips
        O[:, 4] = a_cols[:, 0]
        O[:, 6] = a_cols[:, 1]
        O[:, 5] = b_cols[:, 0]
        O[:, 7] = b_cols[:, 1]
    return np.ascontiguousarray(out)


# revision 18
# speedup vs baseline: 1.1663x; 1.0870x over previous
"""Bass/Trainium2 kernel for nn_Bilinear (out[b,n,i] = enc[b,n,i,:] @ W @ hidden[b,:] + bias).

Sharding: data-parallel over B. 8 cores, one batch element each.

The kernel is HBM-traffic-bound (enc is 32 MiB/core in fp32), so all streamed
operands are cast to fp16 on the host (harness gate is rel_err < 2e-2; fp16
lands ~4e-4): enc 16 MiB + W 2 MiB per core.

With fp16 the DMA stream (~44 us at the observed ~420 GB/s per-core rate)
outpaces what DVE+ScalarE alone can compute (~55 us), so stage 2 is split
across THREE engines. The host lays out each 2 MiB chunk of enc as:
  [ 4 row-major blocks | same-size j-major (transposed) strip of 4 blocks ]
so every DMA is one fully-contiguous 16 KiB run per partition, and:
  - the j-major strip half is reduced on the otherwise-idle TensorE as 8
    PSUM-accumulated [K=128]x[1,512] matmuls against v_col (~2.8 us/chunk),
  - 2 row-major blocks go to DVE custom TENSOR_TENSOR_REDUCE (~1.1 us each),
  - 2 row-major blocks go to DVE fp16 tensor_mul (2x mode, ~0.6 us) +
    ScalarE accumulate-Copy (~1.2 us each).
Per chunk each engine needs ~3 us vs ~4.9 us of DMA: DMA-paced end to end.

Stage 1 (TensorE, fp16): v = W @ h via 16 PSUM-accumulated matmuls behind 8
chunked wt DMAs; v is then both partition-broadcast on the PE (v_rep, for
DVE/ScalarE) and PE-transposed 128 at a time into column form (v_col, the
matmul stationary operand). Bias is folded into the strip PSUM-drain
(activation bias) and added to the block-accumulated columns at the end.

Host-side prep is layout/dtype only (transpose/cast/reshape); all arithmetic
runs on device. The host re-assembles the three output tensors (strip rows,
TTR columns, mul+accum columns) into the full [B, N, I] output.
"""

import numpy as np

B, N, I, H = 8, 64, 128, 1024
P = 128
NI = N * I  # 8192 rows per core
KB = H // P  # 8 k blocks for stage 1
N_CORES = 8
NCH = 8  # stage-2 chunks per core; each covers 8 row blocks (2 MiB fp16)

_NC_CACHE = {}
LAST_RESULTS = None


def _build(ebufs=6):
    import concourse.bacc as bacc
    import concourse.mybir as mybir
    import concourse.tile as tile
    from concourse import dve_ops

    f32 = mybir.dt.float32
    f16 = mybir.dt.float16
    Copy = mybir.ActivationFunctionType.Copy

    nc = bacc.Bacc(
        "TRN2",
        target_bir_lowering=False,
        debug=False,
        num_devices=N_CORES,
    )
    enc = nc.declare_dram_parameter("enc", [P, NCH * 8192], f16, isOutput=False)
    hh = nc.declare_dram_parameter("h", [P, KB], f16, isOutput=False)
    wt = nc.declare_dram_parameter("wt", [P, KB * H], f16, isOutput=False)
    bb = nc.declare_dram_parameter("bias", [1, 1], f32, isOutput=False)
    out_rows = nc.declare_dram_parameter("out_rows", [1, NCH * 512], f32, isOutput=True)
    out_a = nc.declare_dram_parameter("out_a", [P, 2 * NCH], f32, isOutput=True)
    out_b = nc.declare_dram_parameter("out_b", [P, 2 * NCH], f32, isOutput=True)

    with tile.TileContext(nc) as tc:
        with (
            tc.tile_pool(name="const", bufs=1) as const,
            tc.tile_pool(name="tpool", bufs=ebufs) as tpool,
            tc.tile_pool(name="rpool", bufs=ebufs) as rpool,
            tc.tile_pool(name="ppool", bufs=3) as ppool,
            tc.tile_pool(name="vpsum", bufs=1, space="PSUM") as vpsum,
            tc.tile_pool(name="spsum", bufs=3, space="PSUM") as spsum,
        ):
            # ---- stage 1: v[j] = sum_k wt[k,j] h[k] ----
            h_col = const.tile([P, KB], f16)
            nc.sync.dma_start(out=h_col[:], in_=hh[:, :])
            bias_col = const.tile([P, 1], f32)
            nc.sync.dma_start(out=bias_col[:], in_=bb[:, :].to_broadcast((P, 1)))
            bias_one = const.tile([1, 1], f32)
            nc.sync.dma_start(out=bias_one[:], in_=bb[:, :])
            # wt host-packed as [p, kb*H + j] = W.T[kb*128+p, j]; DMA'd in
            # 8 chunks so the stage-1 matmuls pipeline behind the stream
            wt_sb = const.tile([P, KB * H], f16)
            for kb in range(KB):
                nc.sync.dma_start(
                    out=wt_sb[:, kb * H : (kb + 1) * H],
                    in_=wt[:, kb * H : (kb + 1) * H],
                )
            ones = const.tile([1, P], f16)
            nc.vector.memset(ones[:], 1.0)
            id1 = const.tile([1, 1], f16)
            nc.vector.memset(id1[:], 1.0)

            v_flat = const.tile([1, H], f16)
            vps = [
                vpsum.tile([1, 512], f32, name=f"vp{jc}", tag=f"vp{jc}")
                for jc in range(H // 512)
            ]
            for kb in range(KB):
                for jc in range(H // 512):
                    nc.tensor.matmul(
                        vps[jc][:],
                        h_col[:, kb : kb + 1],
                        wt_sb[:, kb * H + jc * 512 : kb * H + (jc + 1) * 512],
                        start=(kb == 0),
                        stop=(kb == KB - 1),
                    )
            for jc in range(H // 512):
                nc.scalar.activation(
                    v_flat[:, jc * 512 : (jc + 1) * 512], vps[jc][:], Copy
                )
            # column form of v for the strip matmuls: v_col[p, jb] = v[jb*128+p]
            # (before the v_rep broadcasts: the PE strip path is the tightest
            # engine, so unblock it first)
            v_col = const.tile([P, KB], f16)
            for jb in range(KB):
                pt = vpsum.tile([P, 1], f16, name=f"pt{jb}", tag="pt")
                nc.tensor.transpose(
                    pt[:], v_flat[:, jb * P : (jb + 1) * P], id1[:]
                )
                nc.scalar.activation(v_col[:, jb : jb + 1], pt[:], Copy)
            # partition-broadcast v on the PE: ones[1,P].T @ v[1,512] -> [P,512]
            v_rep = const.tile([P, H], f16)
            for jc in range(H // 512):
                bc = vpsum.tile([P, 512], f32, name=f"bc{jc}", tag=f"bc{jc}")
                nc.tensor.matmul(
                    bc[:],
                    ones[:],
                    v_flat[:, jc * 512 : (jc + 1) * 512],
                    start=True,
                    stop=True,
                )
                nc.scalar.activation(
                    v_rep[:, jc * 512 : (jc + 1) * 512], bc[:], Copy
                )

            # ---- stage 2 ----
            # Per chunk: DMA the j-major strip half first (PE consumes it),
            # then the row-major half (DVE/ScalarE). The last chunk's DMAs
            # are split in half again to shorten the trailing compute.
            acc_a = const.tile([P, 2 * NCH], f32)
            acc_b = const.tile([P, 2 * NCH], f32)
            dummy_a = const.tile([P, 1], f16)
            Ident = mybir.ActivationFunctionType.Identity

            def rm_block(e_sl, path, col):
                if path == "A":
                    nc.vector._custom_dve(
                        dve_ops.TENSOR_TENSOR_REDUCE,
                        out=dummy_a[:].broadcast_to((P, H)),
                        in0=e_sl,
                        in1=v_rep[:],
                        s0=0.0,
                        s1=1.0,
                        accum_out=acc_a[:, col : col + 1],
                    )
                else:
                    prod = ppool.tile([P, H], f16)
                    nc.vector.tensor_mul(prod[:], e_sl, v_rep[:])
                    nc.scalar.activation(
                        prod[:], prod[:], Copy, accum_out=acc_b[:, col : col + 1]
                    )

            for ci in range(NCH):
                base = ci * 8192
                last = ci == NCH - 1
                ps = spsum.tile([1, 512], f32, name=f"ps{ci}", tag="ps")
                strip = const.tile([1, 512], f32, name=f"st{ci}", tag=f"st{ci}")
                if not last:
                    t = tpool.tile([P, 4096], f16, name=f"t{ci}", tag="t")
                    nc.sync.dma_start(out=t[:], in_=enc[:, base + 4096 : base + 8192])
                    r = rpool.tile([P, 4096], f16, name=f"r{ci}", tag="r")
                    nc.sync.dma_start(out=r[:], in_=enc[:, base : base + 4096])
                    for jb in range(KB):
                        nc.tensor.matmul(
                            ps[:],
                            v_col[:, jb : jb + 1],
                            t[:, jb * 512 : (jb + 1) * 512],
                            start=(jb == 0),
                            stop=(jb == KB - 1),
                        )
                    nc.scalar.activation(strip[:], ps[:], Ident, bias=bias_one[:])
                    nc.gpsimd.dma_start(
                        out=out_rows[:, ci * 512 : (ci + 1) * 512], in_=strip[:]
                    )
                    for slot, path in enumerate(("A", "B", "A", "B")):
                        rm_block(
                            r[:, slot * 1024 : (slot + 1) * 1024],
                            path,
                            2 * ci + slot // 2,
                        )
                else:
                    # tapered final chunk: 4 half-size DMAs
                    ta = tpool.tile([P, 2048], f16, name="t7a", tag="t")
                    nc.sync.dma_start(out=ta[:], in_=enc[:, base + 4096 : base + 6144])
                    tb = tpool.tile([P, 2048], f16, name="t7b", tag="t")
                    nc.sync.dma_start(out=tb[:], in_=enc[:, base + 6144 : base + 8192])
                    ra = rpool.tile([P, 2048], f16, name="r7a", tag="r")
                    nc.sync.dma_start(out=ra[:], in_=enc[:, base : base + 2048])
                    rb = rpool.tile([P, 2048], f16, name="r7b", tag="r")
                    nc.sync.dma_start(out=rb[:], in_=enc[:, base + 2048 : base + 4096])
                    for jb in range(KB):
                        src = ta if jb < 4 else tb
                        off = (jb % 4) * 512
                        nc.tensor.matmul(
                            ps[:],
                            v_col[:, jb : jb + 1],
                            src[:, off : off + 512],
                            start=(jb == 0),
                            stop=(jb == KB - 1),
                        )
                    nc.scalar.activation(strip[:], ps[:], Ident, bias=bias_one[:])
                    nc.gpsimd.dma_start(
                        out=out_rows[:, ci * 512 : (ci + 1) * 512], in_=strip[:]
                    )
                    rm_block(ra[:, 0:1024], "A", 2 * ci)
                    rm_block(ra[:, 1024:2048], "B", 2 * ci)
                    rm_block(rb[:, 0:1024], "A", 2 * ci + 1)
                    rm_block(rb[:, 1024:2048], "B", 2 * ci + 1)

            # bias + writeback of the block-accumulated columns: head columns
            # overlap the final chunk's compute, only the last 2 stay serial
            head = 2 * NCH - 2
            nc.vector.tensor_scalar_add(
                acc_a[:, :head], acc_a[:, :head], bias_col[:]
            )
            nc.gpsimd.dma_start(out=out_a[:, :head], in_=acc_a[:, :head])
            nc.vector.tensor_scalar_add(
                acc_b[:, :head], acc_b[:, :head], bias_col[:]
            )
            nc.gpsimd.dma_start(out=out_b[:, :head], in_=acc_b[:, :head])
            nc.vector.tensor_scalar_add(
                acc_a[:, head:], acc_a[:, head:], bias_col[:]
            )
            nc.gpsimd.dma_start(out=out_a[:, head:], in_=acc_a[:, head:])
            nc.vector.tensor_scalar_add(
                acc_b[:, head:], acc_b[:, head:], bias_col[:]
            )
            nc.gpsimd.dma_start(out=out_b[:, head:], in_=acc_b[:, head:])
    nc.compile()
    return nc


def _get_nc():
    if "nc" not in _NC_CACHE:
        _NC_CACHE["nc"] = _build()
    return _NC_CACHE["nc"]


def _pack_enc(enc16_core):
    """[N*I, H] fp16 -> [P, NCH*8192]: per chunk ci, 4 row-major blocks
    (8ci+4..8ci+7) then the j-major strip of blocks 8ci+0..8ci+3."""
    E = enc16_core.reshape(NCH, 8, P, H)  # [ci, slot, i, j], blk = 8ci+slot
    rm = E[:, 4:8]  # [ci, slot, i, j]
    rm_part = rm.transpose(2, 0, 1, 3).reshape(P, NCH, 4096)  # [i, ci, slot*H+j]
    tr = E[:, 0:4].reshape(NCH, 4, P, KB, P)  # [ci, slot, i, jb, p]
    tr_part = tr.transpose(4, 0, 3, 1, 2).reshape(P, NCH, 4096)  # [p, ci, jb*512+slot*128+i]
    comb = np.concatenate([rm_part, tr_part], axis=2)  # [P, NCH, 8192]
    return np.ascontiguousarray(comb.reshape(P, NCH * 8192))


def kernel(hidden=None, encoder_hiddens=None, input_lengths=None, W=None, b=None):
    global LAST_RESULTS
    from concourse.bass_utils import run_bass_kernel_spmd

    hidden = np.asarray(hidden, dtype=np.float32)
    enc = np.asarray(encoder_hiddens, dtype=np.float32)
    W_ = np.asarray(W, dtype=np.float32)
    b_ = np.asarray(b, dtype=np.float32).reshape(1, 1)
    # wt packed [p, kb*H + j] = W.T[kb*128+p, j]: one contiguous-run DMA
    wt16 = np.ascontiguousarray(
        W_.T.astype(np.float16).reshape(KB, P, H).transpose(1, 0, 2).reshape(P, KB * H)
    )
    enc16 = enc.astype(np.float16)  # [B, N, I, H]

    nc = _get_nc()
    in_maps = []
    for core in range(N_CORES):
        in_maps.append(
            {
                "enc": _pack_enc(enc16[core].reshape(NI, H)),
                "h": np.ascontiguousarray(
                    hidden[core].reshape(KB, P).T.astype(np.float16)
                ),
                "wt": wt16,
                "bias": b_,
            }
        )
    res = run_bass_kernel_spmd(nc, in_maps, core_ids=list(range(N_CORES)))
    LAST_RESULTS = res

    out = np.empty((N_CORES, N, P), dtype=np.float32)
    for c in range(N_CORES):
        r = res.results[c]
        strips = np.asarray(r["out_rows"], dtype=np.float32).reshape(NCH, 4, P)
        a_cols = np.asarray(r["out_a"], dtype=np.float32).T.reshape(NCH, 2, P)
        b_cols = np.asarray(r["out_b"], dtype=np.float32).T.reshape(NCH, 2, P)
        O = out[c].reshape(NCH, 8, P)
        O[:, 0:4] = strips
        O[:, 4] = a_cols[:, 0]
        O[:, 6] = a_cols[:, 1]
        O[:, 5] = b_cols[:, 0]
        O[:, 7] = b_cols[:, 1]
    return np.ascontiguousarray(out)
